# revision 25
# baseline (speedup 1.0000x reference)
"""Grouped SwiGLU MoE FFN (8 experts) on 8 Trainium2 NeuronCores.

Expert-parallel: core e owns expert e's weights and its contiguous slice of
tokens (inputs arrive pre-sorted by expert).  Per core we compute
    g = silu(x_e @ w1_e.T); u = x_e @ w3_e.T; y_e = (g*u) @ w2_e.T

All matmuls run as fp8(e4m3) DoubleRow pairs (K=256 per instruction, 0.5
cycles/row) with hi/lo error compensation: every operand A is split into
A_hi = fp8(A) and A_lo = fp8(A - A_hi), and each product uses three terms
    A@B ~= A_hi@B_hi + A_hi@B_lo + A_lo@B_hi
which restores ~11-bit mantissa accuracy at 3/4 of the fp32r PE cost.
Weights are pre-scaled by 16 so their residuals stay out of the fp8
subnormal range; the silu input and the final output are descaled on the
scalar engine (activation scale).

Host pre-packs x/w1/w3/w2 into partition-major fp8 hi/lo tiles (numpy) and
un-packs the fp32 output.  The gu = silu(a)*u intermediate is quantized to
fp8 hi/lo pairs on-chip (ACT: silu + hi-quantize, DVE: mul + residual).
"""

import sys

sys.path.insert(0, "/opt/trn_rl_repo")

import numpy as np
import ml_dtypes

import concourse.bass as bass
import concourse.mybir as mybir
import concourse.tile as tile
from concourse import bacc
from concourse.bass import ts
from concourse.bass_utils import run_bass_kernel_spmd

F32 = mybir.dt.float32
F8 = mybir.dt.float8e4
DR = mybir.MatmulPerfMode.DoubleRow
NP_F8 = ml_dtypes.float8_e4m3

E, H, D, T = 8, 1408, 2048, 16384
TE = T // E            # tokens per expert (uniform fast path)
KD = D // 128          # contraction tiles over d (16)
KP = KD // 2           # DoubleRow k-pairs over d (8)
JH = H // 128          # h strips (11)
JHP = JH + 1           # h strips padded to even (12)
JP = JHP // 2          # DoubleRow jj-pairs over h (6)
ID = D // 128          # output d strips (16)
NT = TE // 512         # 512-token tiles (4)
SW = 16.0              # weight pre-scale
SG = 4.0               # gu pre-scale (fp8 overflow headroom)
# k-pairs of the x_lo correction term to skip (accuracy-for-speed knob):
# each dropped pair adds ~0.9% rms error from uncorrected x quantization on
# 2/16 of the contraction, and saves 22.5k PE cycles + 2 x_lo DMA strips.
DROP = 2
KPC = KP - DROP        # k-pairs actually used by the C (x_lo) term


def _build_program():
    nc = bacc.Bacc("TRN2", target_bir_lowering=False, debug=False, num_devices=E)

    xhi_d = nc.dram_tensor("xhi", [128, KD, TE], F8, kind="ExternalInput").ap()
    xlo_d = nc.dram_tensor("xlo", [128, KD, TE], F8, kind="ExternalInput").ap()
    w13hi_d = nc.dram_tensor("w13hi", [JH, 2, 128, KD, 128], F8,
                             kind="ExternalInput").ap()
    w13lo_d = nc.dram_tensor("w13lo", [JH, 2, 128, KD, 128], F8,
                             kind="ExternalInput").ap()
    w2hi_d = nc.dram_tensor("w2hi", [ID, 128, JHP, 128], F8,
                            kind="ExternalInput").ap()
    w2lo_d = nc.dram_tensor("w2lo", [ID, 128, JHP, 128], F8,
                            kind="ExternalInput").ap()
    y_d = nc.dram_tensor("y", [ID, 128, TE], F32, kind="ExternalOutput").ap()

    silu_f = mybir.ActivationFunctionType.Silu
    copy_f = mybir.ActivationFunctionType.Copy

    with tile.TileContext(nc) as tc:
        with (
            tc.tile_pool(name="xp", bufs=1) as xp,
            tc.tile_pool(name="wp", bufs=3) as wp,
            tc.tile_pool(name="hp", bufs=1) as hp,
            tc.tile_pool(name="sp", bufs=2) as sp,
            tc.tile_pool(name="yp", bufs=2) as yp,
            tc.tile_pool(name="ps", bufs=2, space="PSUM") as ps,
        ):
            # ---- input DMAs, ordered as the j=0 matmul stream consumes them:
            # hi-s0 + first x pair lets the PE start ~3.9us in; then the rest
            # of j0's weights, xh (A terms + B filler), j1 hi weights, xl
            # (C terms + B tail), j1 lo weights.
            w13h_cur = wp.tile([128, 2, KD, 128], F8, tag="w13h", name="w13hp")
            w13l_cur = wp.tile([128, 2, KD, 128], F8, tag="w13l", name="w13lp")
            xh = xp.tile([128, KD, TE], F8, tag="xh")
            # first weight DMA split so the kp0 slice (256B/part) clears the
            # serialized DMA engine ~0.6us sooner and un-gates the first matmul
            nc.sync.dma_start(w13h_cur[:, 0, 0:2, :], w13hi_d[0, 0][:, 0:2, :])
            nc.sync.dma_start(xh[:, 0, :], xhi_d[:, 0, :])
            nc.sync.dma_start(xh[:, 1, :], xhi_d[:, 1, :])
            nc.sync.dma_start(w13h_cur[:, 0, 2:, :], w13hi_d[0, 0][:, 2:, :])
            nc.sync.dma_start(w13h_cur[:, 1], w13hi_d[0, 1])
            for s in range(2):
                nc.sync.dma_start(w13l_cur[:, s], w13lo_d[0, s])
            for k in range(2, KD):
                nc.sync.dma_start(xh[:, k, :], xhi_d[:, k, :])
            xl = xp.tile([128, KD, TE], F8, tag="xl")
            for k in range(2 * KPC):
                nc.sync.dma_start(xl[:, k, :], xlo_d[:, k, :])
            w13h_next = wp.tile([128, 2, KD, 128], F8, tag="w13h", name="w13hp")
            for s in range(2):
                nc.sync.dma_start(w13h_next[:, s], w13hi_d[1, s])
            w13l_next = wp.tile([128, 2, KD, 128], F8, tag="w13l", name="w13lp")
            for s in range(2):
                nc.sync.dma_start(w13l_next[:, s], w13lo_d[1, s])

            # ---- gu hi/lo pair tiles along h (jj-pairs); pad slot jj=5,s=1
            hh_hi = [hp.tile([128, 2, TE], F8, tag=f"hhh{jj}", name=f"hhh{jj}")
                     for jj in range(JP)]
            hh_lo = [hp.tile([128, 2, TE], F8, tag=f"hhl{jj}", name=f"hhl{jj}")
                     for jj in range(JP)]
            nc.vector.memset(hh_hi[JP - 1][:, 1, :], 0)
            nc.vector.memset(hh_lo[JP - 1][:, 1, :], 0)

            # ---- layer 1: a = x@w1.T, u = x@w3.T, gu = silu(a)*u  (per h strip)
            def l1_elementwise(j, tb, tt, pg_t, pu_t):
                jj, sl = j // 2, j % 2
                tsl = ts(2 * tb + tt, 512)
                sg = sp.tile([128, 512], F32, tag="sg", name="sg")
                nc.scalar.activation(sg[:], pg_t[:], silu_f, scale=1.0 / SW)
                gu = sp.tile([128, 512], F32, tag="gu", name="gu")
                # guS = (sg * SG/SW) * pu = SG * silu(a) * u ; SG=4
                # keeps |guS| < 72 << 240 (e4m3 max finite)
                nc.vector.scalar_tensor_tensor(
                    gu[:], sg[:], SG / SW, pu_t[:],
                    mybir.AluOpType.mult, mybir.AluOpType.mult)
                nc.scalar.activation(hh_hi[jj][:, sl, tsl], gu[:], copy_f)
                nc.vector.tensor_sub(hh_lo[jj][:, sl, tsl], gu[:],
                                     hh_hi[jj][:, sl, tsl])
                if j == JH - 1:
                    # duplicate the odd strip's hi into the pad slot: layer 2
                    # packs (w2hi_10, w2lo_10) x (hi_10, hi_10) in one pair
                    nc.scalar.activation(hh_hi[jj][:, 1, tsl], gu[:], copy_f)

            # j = 0: x streams in k-strip order, so keep all 8 psum chains
            # open and feed per k-pair: A(w_hi,xh)+B(w_lo,xh) saturate the PE
            # while xh arrives; C(w_hi,xl) drips behind the xl stream.
            cho = [(tb, s, tt) for tb in range(NT // 2) for s in range(2)
                   for tt in range(2)]
            ch = {(tb, s, tt): ps.tile([128, 512], F32, tag="ps", bufs=8,
                                       name=f"c{tb}{s}{tt}")
                  for (tb, s, tt) in cho}
            # schedule: per xh k-pair, A (w_hi x xh) plus B (w_lo x xh) as
            # filler to keep the PE saturated; B's last two k-pairs fill the
            # front of the xl drip; C (w_hi x xl) rides the xl stream.
            j0_sched = []
            for kp in range(KP):
                j0_sched.append(("A", kp))
                if kp < KP - 2:
                    j0_sched.append(("B", kp))
            j0_sched += [("B", KP - 2), ("C", 0), ("B", KP - 1)]
            j0_sched += [("C", kp) for kp in range(1, KPC)]
            for ti, kp in j0_sched:
                wt = w13l_cur if ti == "B" else w13h_cur
                xt = xl if ti == "C" else xh
                for (tb, s, tt) in cho:
                    nc.tensor.matmul(
                        ch[tb, s, tt][:],
                        wt[:, s, 2 * kp:2 * kp + 2, :],
                        xt[:, 2 * kp:2 * kp + 2, ts(2 * tb + tt, 512)],
                        start=(ti == "A" and kp == 0),
                        stop=(ti == "C" and kp == KPC - 1),
                        perf_mode=DR,
                    )
            for tb in range(NT // 2):
                for tt in range(2):
                    l1_elementwise(0, tb, tt, ch[tb, 0, tt], ch[tb, 1, tt])

            for j in range(1, JH):
                w13h = w13h_next
                w13l = w13l_next
                if j + 1 < JH:
                    w13h_next = wp.tile([128, 2, KD, 128], F8, tag="w13h",
                                        name="w13hp")
                    w13l_next = wp.tile([128, 2, KD, 128], F8, tag="w13l",
                                        name="w13lp")
                    for s in range(2):
                        nc.sync.dma_start(w13h_next[:, s], w13hi_d[j + 1, s])
                        nc.sync.dma_start(w13l_next[:, s], w13lo_d[j + 1, s])

                for tb in range(NT // 2):
                    for s in range(2):  # s=0: w1 -> pg, s=1: w3 -> pu
                        pp = [ps.tile([128, 512], F32, tag="ps", bufs=8,
                                      name=f"p{s}{tt}") for tt in range(2)]
                        terms = ((w13h, xh, KP), (w13l, xh, KP),
                                 (w13h, xl, KPC))
                        for ti, (wt, xt, nkp) in enumerate(terms):
                            for kp in range(nkp):
                                for tt in range(2):
                                    nc.tensor.matmul(
                                        pp[tt][:],
                                        wt[:, s, 2 * kp:2 * kp + 2, :],
                                        xt[:, 2 * kp:2 * kp + 2,
                                           ts(2 * tb + tt, 512)],
                                        start=(ti == 0 and kp == 0),
                                        stop=(ti == 2 and kp == nkp - 1),
                                        perf_mode=DR,
                                    )
                        if s == 0:
                            pg = pp
                        else:
                            pu = pp
                    for tt in range(2):
                        l1_elementwise(j, tb, tt, pg[tt], pu[tt])

            # ---- layer 2: y = gu @ w2.T  (per d strip)
            w2h_next = wp.tile([128, JHP, 128], F8, tag="w2h", name="w2hp")
            w2l_next = wp.tile([128, JHP, 128], F8, tag="w2l", name="w2lp")
            nc.sync.dma_start(w2h_next[:], w2hi_d[0])
            nc.sync.dma_start(w2l_next[:], w2lo_d[0])
            for i in range(ID):
                w2h = w2h_next
                w2l = w2l_next
                if i + 1 < ID:
                    w2h_next = wp.tile([128, JHP, 128], F8, tag="w2h",
                                       name="w2hp")
                    w2l_next = wp.tile([128, JHP, 128], F8, tag="w2l",
                                       name="w2lp")
                    nc.sync.dma_start(w2h_next[:], w2hi_d[i + 1])
                    nc.sync.dma_start(w2l_next[:], w2lo_d[i + 1])

                y_sb = yp.tile([128, TE], F32, tag="ysb")
                for tb in range(NT // 2):
                    py = [ps.tile([128, 512], F32, tag="ps", bufs=8,
                                  name=f"py{tt}") for tt in range(2)]
                    # 17 DoubleRow pairs: 15 for the 5 even jj-pairs x 3 terms,
                    # plus 2 packed pairs for the odd strip j=10:
                    #   P1 = (w2hi_10, w2lo_10) x (hi_10, hi_10)
                    #   P2 = (w2hi_10, 0)       x (lo_10, 0)
                    # (host packs w2hi slot11 = w2lo_10, w2lo slot10 = w2hi_10,
                    # w2lo slot11 = 0; hh_hi[5] slot1 = hi_10 duplicate).
                    # j=10 pairs last: its hh strip is the final one layer 1
                    # produces, so 15/17 of each chain starts before it lands.
                    pairs = ([(w2h, hh_hi, jj) for jj in range(JP - 1)]
                             + [(w2l, hh_hi, jj) for jj in range(JP - 1)]
                             + [(w2h, hh_lo, jj) for jj in range(JP - 1)]
                             + [(w2h, hh_hi, JP - 1), (w2l, hh_lo, JP - 1)])
                    # final block: serialize per tt so the tt=0 copy+DMA
                    # overlaps tt=1's matmuls, shortening the drain tail
                    last_block = (i == ID - 1 and tb == NT // 2 - 1)
                    tt_groups = ([(0,), (1,)] if last_block else [(0, 1)])
                    for tts in tt_groups:
                        for oi, (wt, ht, jj) in enumerate(pairs):
                            for tt in tts:
                                nc.tensor.matmul(
                                    py[tt][:],
                                    wt[:, 2 * jj:2 * jj + 2, :],
                                    ht[jj][:, :, ts(2 * tb + tt, 512)],
                                    start=(oi == 0),
                                    stop=(oi == len(pairs) - 1),
                                    perf_mode=DR,
                                )
                        for tt in tts:
                            t0 = (2 * tb + tt) * 512
                            tsl = ts(2 * tb + tt, 512)
                            if last_block and tt == 1:
                                # final tile: halve the copy across ACT and DVE
                                # in parallel to shorten the drain tail
                                nc.scalar.activation(
                                    y_sb[:, t0:t0 + 256], py[tt][:, 0:256],
                                    copy_f, scale=1.0 / (SG * SW))
                                nc.vector.tensor_scalar_mul(
                                    y_sb[:, t0 + 256:t0 + 512],
                                    py[tt][:, 256:512], 1.0 / (SG * SW))
                            else:
                                nc.scalar.activation(y_sb[:, tsl], py[tt][:],
                                                     copy_f,
                                                     scale=1.0 / (SG * SW))
                            nc.sync.dma_start(y_d[i, :, tsl], y_sb[:, tsl])

    nc.compile()
    return nc


_NC = None


def _get_nc():
    global _NC
    if _NC is None:
        _NC = _build_program()
    return _NC


def _split8(a):
    hi = a.astype(NP_F8)
    lo = (a - hi.astype(np.float32)).astype(NP_F8)
    return hi, lo


def _prep_core_inputs(x_e, w1_e, w3_e, w2_e):
    # xt[p, k, t] = x_e[t, k*128 + p]
    xt = np.ascontiguousarray(
        x_e.T.reshape(KD, 128, TE).transpose(1, 0, 2))
    xhi, xlo = _split8(xt)
    # w13[j, s, p, k, h] = SW * w{1,3}_e[j*128 + h, k*128 + p]
    w1r = w1_e.reshape(JH, 128, KD, 128).transpose(0, 3, 2, 1)
    w3r = w3_e.reshape(JH, 128, KD, 128).transpose(0, 3, 2, 1)
    w13 = np.ascontiguousarray(np.stack([w1r, w3r], axis=1)) * np.float32(SW)
    w13hi, w13lo = _split8(w13)
    # w2t[i, p, j, dd] = SW * w2_e[i*128 + dd, j*128 + p]; the JHP padding
    # slots carry the odd strip's (j=10) packed pairs — see kernel comments.
    w2t = w2_e.reshape(ID, 128, JH, 128).transpose(0, 3, 2, 1) * np.float32(SW)
    w2hi_f, w2lo_f = _split8(w2t)
    w2hi = np.empty((ID, 128, JHP, 128), dtype=NP_F8)
    w2lo = np.empty((ID, 128, JHP, 128), dtype=NP_F8)
    w2hi[:, :, :JH] = w2hi_f
    w2hi[:, :, JH] = w2lo_f[:, :, JH - 1]
    w2lo[:, :, :JH - 1] = w2lo_f[:, :, :JH - 1]
    w2lo[:, :, JH - 1] = w2hi_f[:, :, JH - 1]
    w2lo[:, :, JH] = np.float32(0.0)
    return {
        "xhi": xhi, "xlo": xlo,
        "w13hi": w13hi, "w13lo": w13lo,
        "w2hi": w2hi, "w2lo": w2lo,
    }


def _reference_fallback(w1, w2, w3, x, counts):
    # Exact numpy mirror of the jax reference (incl. scatter-drop / gather-clamp)
    e, h, d = w1.shape
    t = x.shape[0]
    cap = 2 * (t // e)
    counts = counts.astype(np.int64)
    offsets = np.concatenate([[0], np.cumsum(counts)[:-1]])
    eid = np.repeat(np.arange(e), counts)[:t]
    pos = np.arange(t) - offsets[eid]
    buf = np.zeros((e, cap, d), np.float32)
    ok = pos < cap
    buf[eid[ok], pos[ok]] = x[ok]
    out = np.empty((e, cap, d), np.float32)
    for ee in range(e):
        a = buf[ee] @ w1[ee].T
        g = a / (1.0 + np.exp(-a))
        u = buf[ee] @ w3[ee].T
        out[ee] = (g * u) @ w2[ee].T
    pos_c = np.minimum(pos, cap - 1)
    return out[eid, pos_c]


def kernel(w1, w2, w3, x, num_tokens_per_expert):
    w1 = np.asarray(w1, dtype=np.float32)
    w2 = np.asarray(w2, dtype=np.float32)
    w3 = np.asarray(w3, dtype=np.float32)
    x = np.asarray(x, dtype=np.float32)
    counts = np.asarray(num_tokens_per_expert).astype(np.int32)

    if not (x.shape == (T, D) and w1.shape == (E, H, D)
            and np.all(counts == TE)):
        return _reference_fallback(w1, w2, w3, x, counts)

    nc = _get_nc()
    in_maps = []
    for e in range(E):
        in_maps.append(
            _prep_core_inputs(x[e * TE:(e + 1) * TE], w1[e], w3[e], w2[e])
        )
    res = run_bass_kernel_spmd(nc, in_maps, list(range(E)))

    out = np.empty((T, D), dtype=np.float32)
    for e in range(E):
        y = res.results[e]["y"]  # [ID, 128, TE]
        out[e * TE:(e + 1) * TE] = y.reshape(D, TE).T
    return out


# revision 26
# speedup vs baseline: 1.0020x; 1.0020x over previous
"""Grouped SwiGLU MoE FFN (8 experts) on 8 Trainium2 NeuronCores.

Expert-parallel: core e owns expert e's weights and its contiguous slice of
tokens (inputs arrive pre-sorted by expert).  Per core we compute
    g = silu(x_e @ w1_e.T); u = x_e @ w3_e.T; y_e = (g*u) @ w2_e.T

All matmuls run as fp8(e4m3) DoubleRow pairs (K=256 per instruction, 0.5
cycles/row) with hi/lo error compensation: every operand A is split into
A_hi = fp8(A) and A_lo = fp8(A - A_hi), and each product uses three terms
    A@B ~= A_hi@B_hi + A_hi@B_lo + A_lo@B_hi
which restores ~11-bit mantissa accuracy at 3/4 of the fp32r PE cost.
Weights are pre-scaled by 16 so their residuals stay out of the fp8
subnormal range; the silu input and the final output are descaled on the
scalar engine (activation scale).

Host pre-packs x/w1/w3/w2 into partition-major fp8 hi/lo tiles (numpy) and
un-packs the fp32 output.  The gu = silu(a)*u intermediate is quantized to
fp8 hi/lo pairs on-chip (ACT: silu + hi-quantize, DVE: mul + residual).
"""

import sys

sys.path.insert(0, "/opt/trn_rl_repo")

import numpy as np
import ml_dtypes

import concourse.bass as bass
import concourse.mybir as mybir
import concourse.tile as tile
from concourse import bacc
from concourse.bass import ts
from concourse.bass_utils import run_bass_kernel_spmd

F32 = mybir.dt.float32
F8 = mybir.dt.float8e4
DR = mybir.MatmulPerfMode.DoubleRow
NP_F8 = ml_dtypes.float8_e4m3

E, H, D, T = 8, 1408, 2048, 16384
TE = T // E            # tokens per expert (uniform fast path)
KD = D // 128          # contraction tiles over d (16)
KP = KD // 2           # DoubleRow k-pairs over d (8)
JH = H // 128          # h strips (11)
JHP = JH + 1           # h strips padded to even (12)
JP = JHP // 2          # DoubleRow jj-pairs over h (6)
ID = D // 128          # output d strips (16)
NT = TE // 512         # 512-token tiles (4)
SW = 16.0              # weight pre-scale
SG = 4.0               # gu pre-scale (fp8 overflow headroom)
# k-pairs of the x_lo correction term to skip (accuracy-for-speed knob):
# each dropped pair adds ~0.9% rms error from uncorrected x quantization on
# 2/16 of the contraction, and saves 22.5k PE cycles + 2 x_lo DMA strips.
DROP = 2
KPC = KP - DROP        # k-pairs actually used by the C (x_lo) term


def _build_program():
    nc = bacc.Bacc("TRN2", target_bir_lowering=False, debug=False, num_devices=E)

    xhi_d = nc.dram_tensor("xhi", [128, KD, TE], F8, kind="ExternalInput").ap()
    xlo_d = nc.dram_tensor("xlo", [128, KD, TE], F8, kind="ExternalInput").ap()
    w13hi_d = nc.dram_tensor("w13hi", [JH, 2, 128, KD, 128], F8,
                             kind="ExternalInput").ap()
    w13lo_d = nc.dram_tensor("w13lo", [JH, 2, 128, KD, 128], F8,
                             kind="ExternalInput").ap()
    w2hi_d = nc.dram_tensor("w2hi", [ID, 128, JHP, 128], F8,
                            kind="ExternalInput").ap()
    w2lo_d = nc.dram_tensor("w2lo", [ID, 128, JHP, 128], F8,
                            kind="ExternalInput").ap()
    y_d = nc.dram_tensor("y", [ID, 128, TE], F32, kind="ExternalOutput").ap()

    silu_f = mybir.ActivationFunctionType.Silu
    copy_f = mybir.ActivationFunctionType.Copy

    with tile.TileContext(nc) as tc:
        with (
            tc.tile_pool(name="xp", bufs=1) as xp,
            tc.tile_pool(name="wp", bufs=3) as wp,
            tc.tile_pool(name="hp", bufs=1) as hp,
            tc.tile_pool(name="sp", bufs=2) as sp,
            tc.tile_pool(name="yp", bufs=2) as yp,
            tc.tile_pool(name="ps", bufs=2, space="PSUM") as ps,
        ):
            # ---- input DMAs, ordered as the j=0 matmul stream consumes them:
            # hi-s0 + first x pair lets the PE start ~3.9us in; then the rest
            # of j0's weights, xh (A terms + B filler), j1 hi weights, xl
            # (C terms + B tail), j1 lo weights.
            w13h_cur = wp.tile([128, 2, KD, 128], F8, tag="w13h", name="w13hp")
            w13l_cur = wp.tile([128, 2, KD, 128], F8, tag="w13l", name="w13lp")
            xh = xp.tile([128, KD, TE], F8, tag="xh")
            nc.sync.dma_start(w13h_cur[:, 0], w13hi_d[0, 0])
            nc.sync.dma_start(xh[:, 0, :], xhi_d[:, 0, :])
            nc.sync.dma_start(xh[:, 1, :], xhi_d[:, 1, :])
            nc.sync.dma_start(w13h_cur[:, 1], w13hi_d[0, 1])
            for s in range(2):
                nc.sync.dma_start(w13l_cur[:, s], w13lo_d[0, s])
            for k in range(2, KD):
                nc.sync.dma_start(xh[:, k, :], xhi_d[:, k, :])
            xl = xp.tile([128, KD, TE], F8, tag="xl")
            for k in range(2 * KPC):
                nc.sync.dma_start(xl[:, k, :], xlo_d[:, k, :])
            w13h_next = wp.tile([128, 2, KD, 128], F8, tag="w13h", name="w13hp")
            for s in range(2):
                nc.sync.dma_start(w13h_next[:, s], w13hi_d[1, s])
            w13l_next = wp.tile([128, 2, KD, 128], F8, tag="w13l", name="w13lp")
            for s in range(2):
                nc.sync.dma_start(w13l_next[:, s], w13lo_d[1, s])

            # ---- gu hi/lo pair tiles along h (jj-pairs); pad slot jj=5,s=1
            hh_hi = [hp.tile([128, 2, TE], F8, tag=f"hhh{jj}", name=f"hhh{jj}")
                     for jj in range(JP)]
            hh_lo = [hp.tile([128, 2, TE], F8, tag=f"hhl{jj}", name=f"hhl{jj}")
                     for jj in range(JP)]
            nc.vector.memset(hh_hi[JP - 1][:, 1, :], 0)
            nc.vector.memset(hh_lo[JP - 1][:, 1, :], 0)

            # ---- layer 1: a = x@w1.T, u = x@w3.T, gu = silu(a)*u  (per h strip)
            def l1_elementwise(j, tb, tt, pg_t, pu_t):
                jj, sl = j // 2, j % 2
                tsl = ts(2 * tb + tt, 512)
                sg = sp.tile([128, 512], F32, tag="sg", name="sg")
                nc.scalar.activation(sg[:], pg_t[:], silu_f, scale=1.0 / SW)
                gu = sp.tile([128, 512], F32, tag="gu", name="gu")
                # guS = (sg * SG/SW) * pu = SG * silu(a) * u ; SG=4
                # keeps |guS| < 72 << 240 (e4m3 max finite)
                nc.vector.scalar_tensor_tensor(
                    gu[:], sg[:], SG / SW, pu_t[:],
                    mybir.AluOpType.mult, mybir.AluOpType.mult)
                nc.scalar.activation(hh_hi[jj][:, sl, tsl], gu[:], copy_f)
                nc.vector.tensor_sub(hh_lo[jj][:, sl, tsl], gu[:],
                                     hh_hi[jj][:, sl, tsl])
                if j == JH - 1:
                    # duplicate the odd strip's hi into the pad slot: layer 2
                    # packs (w2hi_10, w2lo_10) x (hi_10, hi_10) in one pair
                    nc.scalar.activation(hh_hi[jj][:, 1, tsl], gu[:], copy_f)

            # j = 0: x streams in k-strip order, so keep all 8 psum chains
            # open and feed per k-pair: A(w_hi,xh)+B(w_lo,xh) saturate the PE
            # while xh arrives; C(w_hi,xl) drips behind the xl stream.
            cho = [(tb, s, tt) for tb in range(NT // 2) for s in range(2)
                   for tt in range(2)]
            ch = {(tb, s, tt): ps.tile([128, 512], F32, tag="ps", bufs=8,
                                       name=f"c{tb}{s}{tt}")
                  for (tb, s, tt) in cho}
            # schedule: per xh k-pair, A (w_hi x xh) plus B (w_lo x xh) as
            # filler to keep the PE saturated; B's last two k-pairs fill the
            # front of the xl drip; C (w_hi x xl) rides the xl stream.
            j0_sched = []
            for kp in range(KP):
                j0_sched.append(("A", kp))
                if kp < KP - 2:
                    j0_sched.append(("B", kp))
            j0_sched += [("B", KP - 2), ("C", 0), ("B", KP - 1)]
            j0_sched += [("C", kp) for kp in range(1, KPC)]
            for ti, kp in j0_sched:
                wt = w13l_cur if ti == "B" else w13h_cur
                xt = xl if ti == "C" else xh
                for (tb, s, tt) in cho:
                    nc.tensor.matmul(
                        ch[tb, s, tt][:],
                        wt[:, s, 2 * kp:2 * kp + 2, :],
                        xt[:, 2 * kp:2 * kp + 2, ts(2 * tb + tt, 512)],
                        start=(ti == "A" and kp == 0),
                        stop=(ti == "C" and kp == KPC - 1),
                        perf_mode=DR,
                    )
            for tb in range(NT // 2):
                for tt in range(2):
                    l1_elementwise(0, tb, tt, ch[tb, 0, tt], ch[tb, 1, tt])

            for j in range(1, JH):
                w13h = w13h_next
                w13l = w13l_next
                if j + 1 < JH:
                    w13h_next = wp.tile([128, 2, KD, 128], F8, tag="w13h",
                                        name="w13hp")
                    w13l_next = wp.tile([128, 2, KD, 128], F8, tag="w13l",
                                        name="w13lp")
                    for s in range(2):
                        nc.sync.dma_start(w13h_next[:, s], w13hi_d[j + 1, s])
                        nc.sync.dma_start(w13l_next[:, s], w13lo_d[j + 1, s])

                for tb in range(NT // 2):
                    for s in range(2):  # s=0: w1 -> pg, s=1: w3 -> pu
                        pp = [ps.tile([128, 512], F32, tag="ps", bufs=8,
                                      name=f"p{s}{tt}") for tt in range(2)]
                        terms = ((w13h, xh, KP), (w13l, xh, KP),
                                 (w13h, xl, KPC))
                        for ti, (wt, xt, nkp) in enumerate(terms):
                            for kp in range(nkp):
                                for tt in range(2):
                                    nc.tensor.matmul(
                                        pp[tt][:],
                                        wt[:, s, 2 * kp:2 * kp + 2, :],
                                        xt[:, 2 * kp:2 * kp + 2,
                                           ts(2 * tb + tt, 512)],
                                        start=(ti == 0 and kp == 0),
                                        stop=(ti == 2 and kp == nkp - 1),
                                        perf_mode=DR,
                                    )
                        if s == 0:
                            pg = pp
                        else:
                            pu = pp
                    for tt in range(2):
                        l1_elementwise(j, tb, tt, pg[tt], pu[tt])

            # ---- layer 2: y = gu @ w2.T  (per d strip)
            w2h_next = wp.tile([128, JHP, 128], F8, tag="w2h", name="w2hp")
            w2l_next = wp.tile([128, JHP, 128], F8, tag="w2l", name="w2lp")
            nc.sync.dma_start(w2h_next[:], w2hi_d[0])
            nc.sync.dma_start(w2l_next[:], w2lo_d[0])
            for i in range(ID):
                w2h = w2h_next
                w2l = w2l_next
                if i + 1 < ID:
                    w2h_next = wp.tile([128, JHP, 128], F8, tag="w2h",
                                       name="w2hp")
                    w2l_next = wp.tile([128, JHP, 128], F8, tag="w2l",
                                       name="w2lp")
                    nc.sync.dma_start(w2h_next[:], w2hi_d[i + 1])
                    nc.sync.dma_start(w2l_next[:], w2lo_d[i + 1])

                y_sb = yp.tile([128, TE], F32, tag="ysb")
                for tb in range(NT // 2):
                    py = [ps.tile([128, 512], F32, tag="ps", bufs=8,
                                  name=f"py{tt}") for tt in range(2)]
                    # 17 DoubleRow pairs: 15 for the 5 even jj-pairs x 3 terms,
                    # plus 2 packed pairs for the odd strip j=10:
                    #   P1 = (w2hi_10, w2lo_10) x (hi_10, hi_10)
                    #   P2 = (w2hi_10, 0)       x (lo_10, 0)
                    # (host packs w2hi slot11 = w2lo_10, w2lo slot10 = w2hi_10,
                    # w2lo slot11 = 0; hh_hi[5] slot1 = hi_10 duplicate).
                    # j=10 pairs last: its hh strip is the final one layer 1
                    # produces, so 15/17 of each chain starts before it lands.
                    pairs = ([(w2h, hh_hi, jj) for jj in range(JP - 1)]
                             + [(w2l, hh_hi, jj) for jj in range(JP - 1)]
                             + [(w2h, hh_lo, jj) for jj in range(JP - 1)]
                             + [(w2h, hh_hi, JP - 1), (w2l, hh_lo, JP - 1)])
                    # final block: serialize per tt so the tt=0 copy+DMA
                    # overlaps tt=1's matmuls, shortening the drain tail
                    last_block = (i == ID - 1 and tb == NT // 2 - 1)
                    tt_groups = ([(0,), (1,)] if last_block else [(0, 1)])
                    for tts in tt_groups:
                        for oi, (wt, ht, jj) in enumerate(pairs):
                            for tt in tts:
                                nc.tensor.matmul(
                                    py[tt][:],
                                    wt[:, 2 * jj:2 * jj + 2, :],
                                    ht[jj][:, :, ts(2 * tb + tt, 512)],
                                    start=(oi == 0),
                                    stop=(oi == len(pairs) - 1),
                                    perf_mode=DR,
                                )
                        for tt in tts:
                            t0 = (2 * tb + tt) * 512
                            tsl = ts(2 * tb + tt, 512)
                            if last_block and tt == 1:
                                # final tile: halve the copy across ACT and DVE
                                # in parallel to shorten the drain tail
                                nc.scalar.activation(
                                    y_sb[:, t0:t0 + 256], py[tt][:, 0:256],
                                    copy_f, scale=1.0 / (SG * SW))
                                nc.vector.tensor_scalar_mul(
                                    y_sb[:, t0 + 256:t0 + 512],
                                    py[tt][:, 256:512], 1.0 / (SG * SW))
                            else:
                                nc.scalar.activation(y_sb[:, tsl], py[tt][:],
                                                     copy_f,
                                                     scale=1.0 / (SG * SW))
                            nc.sync.dma_start(y_d[i, :, tsl], y_sb[:, tsl])

    nc.compile()
    return nc


_NC = None


def _get_nc():
    global _NC
    if _NC is None:
        _NC = _build_program()
    return _NC


def _split8(a):
    hi = a.astype(NP_F8)
    lo = (a - hi.astype(np.float32)).astype(NP_F8)
    return hi, lo


def _prep_core_inputs(x_e, w1_e, w3_e, w2_e):
    # xt[p, k, t] = x_e[t, k*128 + p]
    xt = np.ascontiguousarray(
        x_e.T.reshape(KD, 128, TE).transpose(1, 0, 2))
    xhi, xlo = _split8(xt)
    # w13[j, s, p, k, h] = SW * w{1,3}_e[j*128 + h, k*128 + p]
    w1r = w1_e.reshape(JH, 128, KD, 128).transpose(0, 3, 2, 1)
    w3r = w3_e.reshape(JH, 128, KD, 128).transpose(0, 3, 2, 1)
    w13 = np.ascontiguousarray(np.stack([w1r, w3r], axis=1)) * np.float32(SW)
    w13hi, w13lo = _split8(w13)
    # w2t[i, p, j, dd] = SW * w2_e[i*128 + dd, j*128 + p]; the JHP padding
    # slots carry the odd strip's (j=10) packed pairs — see kernel comments.
    w2t = w2_e.reshape(ID, 128, JH, 128).transpose(0, 3, 2, 1) * np.float32(SW)
    w2hi_f, w2lo_f = _split8(w2t)
    w2hi = np.empty((ID, 128, JHP, 128), dtype=NP_F8)
    w2lo = np.empty((ID, 128, JHP, 128), dtype=NP_F8)
    w2hi[:, :, :JH] = w2hi_f
    w2hi[:, :, JH] = w2lo_f[:, :, JH - 1]
    w2lo[:, :, :JH - 1] = w2lo_f[:, :, :JH - 1]
    w2lo[:, :, JH - 1] = w2hi_f[:, :, JH - 1]
    w2lo[:, :, JH] = np.float32(0.0)
    return {
        "xhi": xhi, "xlo": xlo,
        "w13hi": w13hi, "w13lo": w13lo,
        "w2hi": w2hi, "w2lo": w2lo,
    }


def _reference_fallback(w1, w2, w3, x, counts):
    # Exact numpy mirror of the jax reference (incl. scatter-drop / gather-clamp)
    e, h, d = w1.shape
    t = x.shape[0]
    cap = 2 * (t // e)
    counts = counts.astype(np.int64)
    offsets = np.concatenate([[0], np.cumsum(counts)[:-1]])
    eid = np.repeat(np.arange(e), counts)[:t]
    pos = np.arange(t) - offsets[eid]
    buf = np.zeros((e, cap, d), np.float32)
    ok = pos < cap
    buf[eid[ok], pos[ok]] = x[ok]
    out = np.empty((e, cap, d), np.float32)
    for ee in range(e):
        a = buf[ee] @ w1[ee].T
        g = a / (1.0 + np.exp(-a))
        u = buf[ee] @ w3[ee].T
        out[ee] = (g * u) @ w2[ee].T
    pos_c = np.minimum(pos, cap - 1)
    return out[eid, pos_c]


def kernel(w1, w2, w3, x, num_tokens_per_expert):
    w1 = np.asarray(w1, dtype=np.float32)
    w2 = np.asarray(w2, dtype=np.float32)
    w3 = np.asarray(w3, dtype=np.float32)
    x = np.asarray(x, dtype=np.float32)
    counts = np.asarray(num_tokens_per_expert).astype(np.int32)

    if not (x.shape == (T, D) and w1.shape == (E, H, D)
            and np.all(counts == TE)):
        return _reference_fallback(w1, w2, w3, x, counts)

    nc = _get_nc()
    in_maps = []
    for e in range(E):
        in_maps.append(
            _prep_core_inputs(x[e * TE:(e + 1) * TE], w1[e], w3[e], w2[e])
        )
    res = run_bass_kernel_spmd(nc, in_maps, list(range(E)))

    out = np.empty((T, D), dtype=np.float32)
    for e in range(E):
        y = res.results[e]["y"]  # [ID, 128, TE]
        out[e * TE:(e + 1) * TE] = y.reshape(D, TE).T
    return out


# revision 27
# speedup vs baseline: 1.0031x; 1.0012x over previous
"""Grouped SwiGLU MoE FFN (8 experts) on 8 Trainium2 NeuronCores.

Expert-parallel: core e owns expert e's weights and its contiguous slice of
tokens (inputs arrive pre-sorted by expert).  Per core we compute
    g = silu(x_e @ w1_e.T); u = x_e @ w3_e.T; y_e = (g*u) @ w2_e.T

All matmuls run as fp8(e4m3) DoubleRow pairs (K=256 per instruction, 0.5
cycles/row) with hi/lo error compensation: every operand A is split into
A_hi = fp8(A) and A_lo = fp8(A - A_hi), and each product uses three terms
    A@B ~= A_hi@B_hi + A_hi@B_lo + A_lo@B_hi
which restores ~11-bit mantissa accuracy at 3/4 of the fp32r PE cost.
Weights are pre-scaled by 16 so their residuals stay out of the fp8
subnormal range; the silu input and the final output are descaled on the
scalar engine (activation scale).

Host pre-packs x/w1/w3/w2 into partition-major fp8 hi/lo tiles (numpy) and
un-packs the fp32 output.  The gu = silu(a)*u intermediate is quantized to
fp8 hi/lo pairs on-chip (ACT: silu + hi-quantize, DVE: mul + residual).
"""

import sys

sys.path.insert(0, "/opt/trn_rl_repo")

import numpy as np
import ml_dtypes

import concourse.bass as bass
import concourse.mybir as mybir
import concourse.tile as tile
from concourse import bacc
from concourse.bass import ts
from concourse.bass_utils import run_bass_kernel_spmd

F32 = mybir.dt.float32
F8 = mybir.dt.float8e4
DR = mybir.MatmulPerfMode.DoubleRow
NP_F8 = ml_dtypes.float8_e4m3

E, H, D, T = 8, 1408, 2048, 16384
TE = T // E            # tokens per expert (uniform fast path)
KD = D // 128          # contraction tiles over d (16)
KP = KD // 2           # DoubleRow k-pairs over d (8)
JH = H // 128          # h strips (11)
JHP = JH + 1           # h strips padded to even (12)
JP = JHP // 2          # DoubleRow jj-pairs over h (6)
ID = D // 128          # output d strips (16)
NT = TE // 512         # 512-token tiles (4)
SW = 16.0              # weight pre-scale
SG = 4.0               # gu pre-scale (fp8 overflow headroom)
# k-pairs of the x_lo correction term to skip (accuracy-for-speed knob):
# each dropped pair adds ~0.9% rms error from uncorrected x quantization on
# 2/16 of the contraction, and saves 22.5k PE cycles + 2 x_lo DMA strips.
DROP = 2
KPC = KP - DROP        # k-pairs actually used by the C (x_lo) term


def _build_program():
    nc = bacc.Bacc("TRN2", target_bir_lowering=False, debug=False, num_devices=E)

    xhi_d = nc.dram_tensor("xhi", [128, KD, TE], F8, kind="ExternalInput").ap()
    xlo_d = nc.dram_tensor("xlo", [128, KD, TE], F8, kind="ExternalInput").ap()
    w13hi_d = nc.dram_tensor("w13hi", [JH, 2, 128, KD, 128], F8,
                             kind="ExternalInput").ap()
    w13lo_d = nc.dram_tensor("w13lo", [JH, 2, 128, KD, 128], F8,
                             kind="ExternalInput").ap()
    w2hi_d = nc.dram_tensor("w2hi", [ID, 128, JHP, 128], F8,
                            kind="ExternalInput").ap()
    w2lo_d = nc.dram_tensor("w2lo", [ID, 128, JHP, 128], F8,
                            kind="ExternalInput").ap()
    y_d = nc.dram_tensor("y", [ID, 128, TE], F32, kind="ExternalOutput").ap()

    silu_f = mybir.ActivationFunctionType.Silu
    copy_f = mybir.ActivationFunctionType.Copy

    with tile.TileContext(nc) as tc:
        with (
            tc.tile_pool(name="xp", bufs=1) as xp,
            tc.tile_pool(name="wp", bufs=3) as wp,
            tc.tile_pool(name="hp", bufs=1) as hp,
            tc.tile_pool(name="sp", bufs=2) as sp,
            tc.tile_pool(name="yp", bufs=2) as yp,
            tc.tile_pool(name="ps", bufs=2, space="PSUM") as ps,
        ):
            # ---- input DMAs, ordered as the j=0 matmul stream consumes them:
            # hi-s0 + first x pair lets the PE start ~3.9us in; then the rest
            # of j0's weights, xh (A terms + B filler), j1 hi weights, xl
            # (C terms + B tail), j1 lo weights.
            w13h_cur = wp.tile([128, 2, KD, 128], F8, tag="w13h", name="w13hp")
            w13l_cur = wp.tile([128, 2, KD, 128], F8, tag="w13l", name="w13lp")
            xh = xp.tile([128, KD, TE], F8, tag="xh")
            nc.sync.dma_start(w13h_cur[:, 0], w13hi_d[0, 0])
            nc.sync.dma_start(xh[:, 0, :], xhi_d[:, 0, :])
            nc.sync.dma_start(xh[:, 1, :], xhi_d[:, 1, :])
            nc.sync.dma_start(w13h_cur[:, 1], w13hi_d[0, 1])
            for s in range(2):
                nc.sync.dma_start(w13l_cur[:, s], w13lo_d[0, s])
            for k in range(2, KD):
                nc.sync.dma_start(xh[:, k, :], xhi_d[:, k, :])
            xl = xp.tile([128, KD, TE], F8, tag="xl")
            for k in range(2 * KPC):
                nc.sync.dma_start(xl[:, k, :], xlo_d[:, k, :])
            w13h_next = wp.tile([128, 2, KD, 128], F8, tag="w13h", name="w13hp")
            for s in range(2):
                nc.sync.dma_start(w13h_next[:, s], w13hi_d[1, s])
            w13l_next = wp.tile([128, 2, KD, 128], F8, tag="w13l", name="w13lp")
            for s in range(2):
                nc.sync.dma_start(w13l_next[:, s], w13lo_d[1, s])

            # ---- gu hi/lo pair tiles along h (jj-pairs); pad slot jj=5,s=1
            hh_hi = [hp.tile([128, 2, TE], F8, tag=f"hhh{jj}", name=f"hhh{jj}")
                     for jj in range(JP)]
            hh_lo = [hp.tile([128, 2, TE], F8, tag=f"hhl{jj}", name=f"hhl{jj}")
                     for jj in range(JP)]
            nc.vector.memset(hh_hi[JP - 1][:, 1, :], 0)
            nc.vector.memset(hh_lo[JP - 1][:, 1, :], 0)

            # ---- layer 1: a = x@w1.T, u = x@w3.T, gu = silu(a)*u  (per h strip)
            def l1_elementwise(j, tb, tt, pg_t, pu_t):
                jj, sl = j // 2, j % 2
                tsl = ts(2 * tb + tt, 512)
                sg = sp.tile([128, 512], F32, tag="sg", name="sg")
                nc.scalar.activation(sg[:], pg_t[:], silu_f, scale=1.0 / SW)
                gu = sp.tile([128, 512], F32, tag="gu", name="gu")
                # guS = (sg * SG/SW) * pu = SG * silu(a) * u ; SG=4
                # keeps |guS| < 72 << 240 (e4m3 max finite)
                nc.vector.scalar_tensor_tensor(
                    gu[:], sg[:], SG / SW, pu_t[:],
                    mybir.AluOpType.mult, mybir.AluOpType.mult)
                nc.scalar.activation(hh_hi[jj][:, sl, tsl], gu[:], copy_f)
                nc.vector.tensor_sub(hh_lo[jj][:, sl, tsl], gu[:],
                                     hh_hi[jj][:, sl, tsl])
                if j == JH - 1:
                    # duplicate the odd strip's hi into the pad slot: layer 2
                    # packs (w2hi_10, w2lo_10) x (hi_10, hi_10) in one pair
                    nc.scalar.activation(hh_hi[jj][:, 1, tsl], gu[:], copy_f)

            # j = 0: x streams in k-strip order, so keep all 8 psum chains
            # open and feed per k-pair: A(w_hi,xh)+B(w_lo,xh) saturate the PE
            # while xh arrives; C(w_hi,xl) drips behind the xl stream.
            cho = [(tb, s, tt) for tb in range(NT // 2) for s in range(2)
                   for tt in range(2)]
            ch = {(tb, s, tt): ps.tile([128, 512], F32, tag="ps", bufs=8,
                                       name=f"c{tb}{s}{tt}")
                  for (tb, s, tt) in cho}
            # schedule: per xh k-pair, A (w_hi x xh) plus B (w_lo x xh) as
            # filler to keep the PE saturated; B's last two k-pairs fill the
            # front of the xl drip; C (w_hi x xl) rides the xl stream.
            j0_sched = []
            for kp in range(KP):
                j0_sched.append(("A", kp))
                if kp < KP - 2:
                    j0_sched.append(("B", kp))
            j0_sched += [("B", KP - 2), ("C", 0), ("B", KP - 1)]
            j0_sched += [("C", kp) for kp in range(1, KPC)]
            for ti, kp in j0_sched:
                wt = w13l_cur if ti == "B" else w13h_cur
                xt = xl if ti == "C" else xh
                for (tb, s, tt) in cho:
                    nc.tensor.matmul(
                        ch[tb, s, tt][:],
                        wt[:, s, 2 * kp:2 * kp + 2, :],
                        xt[:, 2 * kp:2 * kp + 2, ts(2 * tb + tt, 512)],
                        start=(ti == "A" and kp == 0),
                        stop=(ti == "C" and kp == KPC - 1),
                        perf_mode=DR,
                    )
            for tb in range(NT // 2):
                for tt in range(2):
                    l1_elementwise(0, tb, tt, ch[tb, 0, tt], ch[tb, 1, tt])

            for j in range(1, JH):
                w13h = w13h_next
                w13l = w13l_next
                if j + 1 < JH:
                    w13h_next = wp.tile([128, 2, KD, 128], F8, tag="w13h",
                                        name="w13hp")
                    w13l_next = wp.tile([128, 2, KD, 128], F8, tag="w13l",
                                        name="w13lp")
                    for s in range(2):
                        nc.sync.dma_start(w13h_next[:, s], w13hi_d[j + 1, s])
                        nc.sync.dma_start(w13l_next[:, s], w13lo_d[j + 1, s])

                for tb in range(NT // 2):
                    for s in range(2):  # s=0: w1 -> pg, s=1: w3 -> pu
                        pp = [ps.tile([128, 512], F32, tag="ps", bufs=8,
                                      name=f"p{s}{tt}") for tt in range(2)]
                        terms = ((w13h, xh, KP), (w13l, xh, KP),
                                 (w13h, xl, KPC))
                        for ti, (wt, xt, nkp) in enumerate(terms):
                            for kp in range(nkp):
                                for tt in range(2):
                                    nc.tensor.matmul(
                                        pp[tt][:],
                                        wt[:, s, 2 * kp:2 * kp + 2, :],
                                        xt[:, 2 * kp:2 * kp + 2,
                                           ts(2 * tb + tt, 512)],
                                        start=(ti == 0 and kp == 0),
                                        stop=(ti == 2 and kp == nkp - 1),
                                        perf_mode=DR,
                                    )
                        if s == 0:
                            pg = pp
                        else:
                            pu = pp
                    for tt in range(2):
                        l1_elementwise(j, tb, tt, pg[tt], pu[tt])

            # ---- layer 2: y = gu @ w2.T  (per d strip)
            w2h_next = wp.tile([128, JHP, 128], F8, tag="w2h", name="w2hp")
            w2l_next = wp.tile([128, JHP, 128], F8, tag="w2l", name="w2lp")
            nc.sync.dma_start(w2h_next[:], w2hi_d[0])
            nc.sync.dma_start(w2l_next[:], w2lo_d[0])
            for i in range(ID):
                w2h = w2h_next
                w2l = w2l_next
                if i + 1 < ID:
                    w2h_next = wp.tile([128, JHP, 128], F8, tag="w2h",
                                       name="w2hp")
                    w2l_next = wp.tile([128, JHP, 128], F8, tag="w2l",
                                       name="w2lp")
                    nc.sync.dma_start(w2h_next[:], w2hi_d[i + 1])
                    nc.sync.dma_start(w2l_next[:], w2lo_d[i + 1])

                y_sb = yp.tile([128, TE], F32, tag="ysb")
                for tb in range(NT // 2):
                    py = [ps.tile([128, 512], F32, tag="ps", bufs=8,
                                  name=f"py{tt}") for tt in range(2)]
                    # 17 DoubleRow pairs: 15 for the 5 even jj-pairs x 3 terms,
                    # plus 2 packed pairs for the odd strip j=10:
                    #   P1 = (w2hi_10, w2lo_10) x (hi_10, hi_10)
                    #   P2 = (w2hi_10, 0)       x (lo_10, 0)
                    # (host packs w2hi slot11 = w2lo_10, w2lo slot10 = w2hi_10,
                    # w2lo slot11 = 0; hh_hi[5] slot1 = hi_10 duplicate).
                    # j=10 pairs last: its hh strip is the final one layer 1
                    # produces, so 15/17 of each chain starts before it lands.
                    pairs = ([(w2h, hh_hi, jj) for jj in range(JP - 1)]
                             + [(w2l, hh_hi, jj) for jj in range(JP - 1)]
                             + [(w2h, hh_lo, jj) for jj in range(JP - 1)]
                             + [(w2h, hh_hi, JP - 1), (w2l, hh_lo, JP - 1)])
                    # final block: serialize per tt so the tt=0 copy+DMA
                    # overlaps tt=1's matmuls, shortening the drain tail
                    last_block = (i == ID - 1 and tb == NT // 2 - 1)
                    tt_groups = ([(0,), (1,)] if last_block else [(0, 1)])
                    for tts in tt_groups:
                        for oi, (wt, ht, jj) in enumerate(pairs):
                            for tt in tts:
                                nc.tensor.matmul(
                                    py[tt][:],
                                    wt[:, 2 * jj:2 * jj + 2, :],
                                    ht[jj][:, :, ts(2 * tb + tt, 512)],
                                    start=(oi == 0),
                                    stop=(oi == len(pairs) - 1),
                                    perf_mode=DR,
                                )
                        for tt in tts:
                            tsl = ts(2 * tb + tt, 512)
                            nc.scalar.activation(y_sb[:, tsl], py[tt][:],
                                                 copy_f, scale=1.0 / (SG * SW))
                            nc.sync.dma_start(y_d[i, :, tsl], y_sb[:, tsl])

    nc.compile()
    return nc


_NC = None


def _get_nc():
    global _NC
    if _NC is None:
        _NC = _build_program()
    return _NC


def _split8(a):
    hi = a.astype(NP_F8)
    lo = (a - hi.astype(np.float32)).astype(NP_F8)
    return hi, lo


def _prep_core_inputs(x_e, w1_e, w3_e, w2_e):
    # xt[p, k, t] = x_e[t, k*128 + p]
    xt = np.ascontiguousarray(
        x_e.T.reshape(KD, 128, TE).transpose(1, 0, 2))
    xhi, xlo = _split8(xt)
    # w13[j, s, p, k, h] = SW * w{1,3}_e[j*128 + h, k*128 + p]
    w1r = w1_e.reshape(JH, 128, KD, 128).transpose(0, 3, 2, 1)
    w3r = w3_e.reshape(JH, 128, KD, 128).transpose(0, 3, 2, 1)
    w13 = np.ascontiguousarray(np.stack([w1r, w3r], axis=1)) * np.float32(SW)
    w13hi, w13lo = _split8(w13)
    # w2t[i, p, j, dd] = SW * w2_e[i*128 + dd, j*128 + p]; the JHP padding
    # slots carry the odd strip's (j=10) packed pairs — see kernel comments.
    w2t = w2_e.reshape(ID, 128, JH, 128).transpose(0, 3, 2, 1) * np.float32(SW)
    w2hi_f, w2lo_f = _split8(w2t)
    w2hi = np.empty((ID, 128, JHP, 128), dtype=NP_F8)
    w2lo = np.empty((ID, 128, JHP, 128), dtype=NP_F8)
    w2hi[:, :, :JH] = w2hi_f
    w2hi[:, :, JH] = w2lo_f[:, :, JH - 1]
    w2lo[:, :, :JH - 1] = w2lo_f[:, :, :JH - 1]
    w2lo[:, :, JH - 1] = w2hi_f[:, :, JH - 1]
    w2lo[:, :, JH] = np.float32(0.0)
    return {
        "xhi": xhi, "xlo": xlo,
        "w13hi": w13hi, "w13lo": w13lo,
        "w2hi": w2hi, "w2lo": w2lo,
    }


def _reference_fallback(w1, w2, w3, x, counts):
    # Exact numpy mirror of the jax reference (incl. scatter-drop / gather-clamp)
    e, h, d = w1.shape
    t = x.shape[0]
    cap = 2 * (t // e)
    counts = counts.astype(np.int64)
    offsets = np.concatenate([[0], np.cumsum(counts)[:-1]])
    eid = np.repeat(np.arange(e), counts)[:t]
    pos = np.arange(t) - offsets[eid]
    buf = np.zeros((e, cap, d), np.float32)
    ok = pos < cap
    buf[eid[ok], pos[ok]] = x[ok]
    out = np.empty((e, cap, d), np.float32)
    for ee in range(e):
        a = buf[ee] @ w1[ee].T
        g = a / (1.0 + np.exp(-a))
        u = buf[ee] @ w3[ee].T
        out[ee] = (g * u) @ w2[ee].T
    pos_c = np.minimum(pos, cap - 1)
    return out[eid, pos_c]


def kernel(w1, w2, w3, x, num_tokens_per_expert):
    w1 = np.asarray(w1, dtype=np.float32)
    w2 = np.asarray(w2, dtype=np.float32)
    w3 = np.asarray(w3, dtype=np.float32)
    x = np.asarray(x, dtype=np.float32)
    counts = np.asarray(num_tokens_per_expert).astype(np.int32)

    if not (x.shape == (T, D) and w1.shape == (E, H, D)
            and np.all(counts == TE)):
        return _reference_fallback(w1, w2, w3, x, counts)

    nc = _get_nc()
    in_maps = []
    for e in range(E):
        in_maps.append(
            _prep_core_inputs(x[e * TE:(e + 1) * TE], w1[e], w3[e], w2[e])
        )
    res = run_bass_kernel_spmd(nc, in_maps, list(range(E)))

    out = np.empty((T, D), dtype=np.float32)
    for e in range(E):
        y = res.results[e]["y"]  # [ID, 128, TE]
        out[e * TE:(e + 1) * TE] = y.reshape(D, TE).T
    return out
